# revision 6
# baseline (speedup 1.0000x reference)
"""Trainium2 Bass kernel for nn_ConvAttentionBlock (B=8, H=W=64, C=HC=128).

Sharding: data-parallel over batch — each of the 8 NeuronCores runs the full
attention block for one [64*64, 128] image.

Per-core algorithm (NT=4096 tokens, C=HC=128), restructured from the
baseline to balance engines (PE is the only near-saturated engine):

  qs = kappa*(x@wq + bq) ; k = x@wk + bk ; v = x@wv   (bv folded into cvec)
  S^T[keys, q] = k qs^T            (scores pre-scaled by kappa = 128*log2(e)
                                    so the DVE exp trick needs no multiply)
  E = exp(s - 50) as bf16, computed per 128-key chunk on a rotating engine:
    - ACT: exp activation, scale=1/kappa, bias=-50          (3 of 4 chunks)
    - DVE: Schraudolph bit-trick: i16 = max(S_scaled + C, 0) rounded,
      bitcast to bf16 == 2^(log2e*(s-50)) with ~3% PWL error (1 of 4 chunks)
  l_q = sum_keys E: bf16 running accumulation (DVE) then one GPSIMD
      partition-axis (axis=C) reduce per 1024-q stripe + DRAM-bounce
      scatter [1,1024] -> [128,8] to get per-token layout.
  attT[HC, q] += v_chunk^T E        (PSUM accumulation over 32 key chunks)
  out = relu(x + (attT^T @ wp)/l + cvec),  cvec = bv@wp + bp  (host-folded)

Exp tiles are [128, 1024] (one ACT/DVE instruction per key-chunk x stripe)
to amortize per-instruction overhead; S PSUM tiles are [128,1024] (2 banks,
3 bufs) + attT accumulator [128,1024] (2 banks) = 8 banks.
"""

import numpy as np

try:
    import concourse.bass as bass
except ImportError:  # pragma: no cover - fallback for bare containers
    import sys
    for p in ("/opt/trn_rl_repo", "/root/.axon_site/_ro/trn_rl_repo"):
        if p not in sys.path:
            sys.path.insert(0, p)
    import concourse.bass as bass

import concourse.mybir as mybir
import concourse.tile as tile
from concourse import bacc, bass_isa
from concourse.bass import ts
from concourse.bass_utils import run_bass_kernel_spmd
from concourse.masks import make_identity

F32 = mybir.dt.float32
F32R = mybir.dt.float32r
BF16 = mybir.dt.bfloat16
I16 = mybir.dt.int16
AF = mybir.ActivationFunctionType
OP = mybir.AluOpType
AX = mybir.AxisListType

B, H, W = 8, 64, 64
NT = H * W            # 4096 tokens per image
C = HC = 128
P = 128
CSHIFT = 50.0         # exp(s - CSHIFT): scores empirically within [-84, 94]
STRIPE = 1024         # q-stripe: S psum [128,1024] (2 banks) x 3 bufs
                      # + attT psum [128,1024] (2 banks) = 8 banks
KAPPA = 128.0 * np.log2(np.e)          # score pre-scale for i16 exp trick
SCHC = 128.0 * (127.0 - CSHIFT * np.log2(np.e)) - 8.0   # schraudolph bias


def build(NT=NT, stripe=STRIPE, s_bufs=3, e_bufs=6, reps=1, hw_loop=None,
          dve_every=4):
    NCH = NT // P          # 128-token chunks
    NG = NT // 512         # 512-token groups
    n_stripes = NT // stripe
    JC = stripe // 512     # 512-wide q chunks per stripe
    CPS = stripe // P      # 128-token chunks per stripe

    nc = bacc.Bacc(None, target_bir_lowering=False)
    x_d = nc.dram_tensor("x", [NT, C], F32, kind="ExternalInput")
    wq_d = nc.dram_tensor("wq", [C, HC], F32R, kind="ExternalInput")
    wk_d = nc.dram_tensor("wk", [C, HC], F32R, kind="ExternalInput")
    wv_d = nc.dram_tensor("wv", [C, HC], F32R, kind="ExternalInput")
    wp_d = nc.dram_tensor("wp", [HC, C], F32, kind="ExternalInput")
    bq_d = nc.dram_tensor("bq", [HC, 1], F32, kind="ExternalInput")
    bk_d = nc.dram_tensor("bk", [HC, 1], F32, kind="ExternalInput")
    cv_d = nc.dram_tensor("cvec", [1, C], F32, kind="ExternalInput")
    out_d = nc.dram_tensor("out", [NT, C], F32, kind="ExternalOutput")
    l_dram = nc.dram_tensor("lscratch", [NT], F32, kind="ExternalOutput")

    x_src = x_d[:].rearrange("(n p) c -> p n c", p=P)  # [128, NCH, C]

    with tile.TileContext(nc) as tc:
        with (
            tc.tile_pool(name="consts", bufs=1) as consts,
            tc.tile_pool(name="big", bufs=1) as big,
        ):
            # ---- constants ----
            wq_t = consts.tile([C, HC], F32R)
            wk_t = consts.tile([C, HC], F32R)
            wv_t = consts.tile([C, HC], F32R)
            wp_t = consts.tile([HC, C], F32)
            wp_bf = consts.tile([HC, C], BF16)
            bq_t = consts.tile([HC, 1], F32)
            bqs_t = consts.tile([HC, 1], F32)
            bk_t = consts.tile([HC, 1], F32)
            ident = consts.tile([P, P], F32)
            cvec_bc = consts.tile([P, C], F32)
            nshift = consts.tile([P, 1], F32)
            nc.vector.memset(nshift[:], -CSHIFT)
            nc.sync.dma_start(wq_t[:], wq_d[:])
            nc.sync.dma_start(wk_t[:], wk_d[:])
            nc.sync.dma_start(wv_t[:], wv_d[:])
            nc.sync.dma_start(wp_t[:], wp_d[:])
            nc.sync.dma_start(bq_t[:], bq_d[:])
            nc.sync.dma_start(bk_t[:], bk_d[:])
            nc.sync.dma_start(cvec_bc[:], cv_d[:].to_broadcast([P, C]))
            make_identity(nc, ident[:])
            nc.vector.tensor_copy(wp_bf[:], wp_t[:])
            nc.scalar.mul(bqs_t[:], bq_t[:], KAPPA)

            # ---- persistent tiles ----
            x_nat = big.tile([P, NCH, C], F32)     # becomes x + cvec later
            xT = big.tile([C, NCH, P], F32R)       # x transposed [C, token]
            qTs = big.tile([HC, NT], F32R)         # kappa * (x@wq + bq)
            kT = big.tile([HC, NT], F32R)
            v = big.tile([P, NCH, HC], BF16)       # [token, HC]
            attT = big.tile([HC, NT], BF16)
            acc = big.tile([P, NT], BF16)          # exp partials over chunks
            l_bc = big.tile([P, STRIPE], F32)      # sum over keys, replicated
            l32 = big.tile([P, NCH], F32)          # per-token l
            recip_l = big.tile([P, NCH], F32)

            def emit(rep):
                # ---- Phase A/B: load, transpose, QKV ----
                with tc.tile_pool(name="ps_ab", bufs=3, space="PSUM") as ps_ab:
                    for g in range(NG):
                        nc.sync.dma_start(
                            x_nat[:, ts(g, 4), :], x_src[:, ts(g, 4), :]
                        )
                        for u in range(4):
                            i = g * 4 + u
                            tp = ps_ab.tile([P, P], F32, tag="tp")
                            nc.tensor.transpose(tp[:], x_nat[:, i, :], ident[:])
                            nc.vector.tensor_copy(xT[:, i, :], tp[:])
                        qp = ps_ab.tile([P, 512], F32, tag="qkv")
                        nc.tensor.matmul(qp[:], wq_t[:], xT[:, ts(g, 4), :],
                                         start=True, stop=True)
                        nc.scalar.activation(qTs[:, ts(g, 512)], qp[:],
                                             AF.Identity, bias=bqs_t[:, 0:1],
                                             scale=KAPPA)
                        kp = ps_ab.tile([P, 512], F32, tag="qkv")
                        nc.tensor.matmul(kp[:], wk_t[:], xT[:, ts(g, 4), :],
                                         start=True, stop=True)
                        nc.scalar.activation(kT[:, ts(g, 512)], kp[:],
                                             AF.Identity, bias=bk_t[:, 0:1],
                                             scale=1.0)
                        for u in range(4):
                            i = g * 4 + u
                            vp = ps_ab.tile([P, HC], F32, tag="tp")
                            nc.tensor.matmul(vp[:], xT[:, i, :], wv_t[:],
                                             start=True, stop=True)
                            nc.vector.tensor_copy(v[:, i, :], vp[:])
                        for u in range(4):
                            i = g * 4 + u
                            nc.gpsimd.tensor_tensor(
                                x_nat[:, i, :], x_nat[:, i, :], cvec_bc[:],
                                OP.add)

                # ---- Phase C: attention, per stripe (software-pipelined:
                # S matmuls run LA chunks ahead of the exp->AV consumers so
                # the PE never head-of-line blocks on the exp engines) ----
                LA = 2
                with (
                    tc.tile_pool(name="ps_att", bufs=1, space="PSUM") as ps_att,
                    tc.tile_pool(name="ps_s", bufs=s_bufs, space="PSUM") as ps_s,
                    tc.tile_pool(name="epool", bufs=e_bufs) as epool,
                ):
                    for s in range(n_stripes):
                        qsl = slice(s * stripe, (s + 1) * stripe)
                        att_ps = ps_att.tile([HC, stripe], F32, tag="att")
                        sp_live = {}
                        for step in range(NCH + LA):
                            if step < NCH:
                                kt = step
                                sp = ps_s.tile([P, stripe], F32, tag="s")
                                sp_live[kt] = sp
                                for jc in range(JC):
                                    nc.tensor.matmul(
                                        sp[:, ts(jc, 512)], kT[:, ts(kt, P)],
                                        qTs[:, s * stripe + jc * 512:
                                                s * stripe + (jc + 1) * 512],
                                        start=True, stop=True)
                            if step >= LA:
                                kt = step - LA
                                sp = sp_live.pop(kt)
                                E = epool.tile([P, stripe], BF16, tag="e")
                                if kt % dve_every == dve_every - 1:
                                    # Schraudolph: bf16 bits =
                                    #   max(round(s*kappa + C), 0)
                                    nc.vector.tensor_scalar(
                                        E[:].bitcast(I16), sp[:], SCHC, 0.0,
                                        OP.add, OP.max)
                                else:
                                    nc.scalar.activation(E[:], sp[:], AF.Exp,
                                                         bias=nshift[:, 0:1],
                                                         scale=1.0 / KAPPA)
                                if kt == 0:
                                    nc.vector.tensor_copy(acc[:, qsl], E[:])
                                else:
                                    nc.vector.tensor_tensor(acc[:, qsl],
                                                            acc[:, qsl], E[:],
                                                            OP.add)
                                for jc in range(JC):
                                    nc.tensor.matmul(
                                        att_ps[:, ts(jc, 512)], v[:, kt, :],
                                        E[:, ts(jc, 512)],
                                        start=(kt == 0), stop=(kt == NCH - 1))
                        nc.scalar.copy(attT[:, qsl], att_ps[:])

                        # l: partition-reduce (gpsimd), then DRAM-bounce
                        # scatter [1, stripe] -> [128, CPS] token layout
                        nc.gpsimd.partition_all_reduce(
                            l_bc[:], acc[:, qsl], channels=P,
                            reduce_op=bass_isa.ReduceOp.add)
                        nc.sync.dma_start(l_dram[qsl], l_bc[0:1, :])
                        nc.sync.dma_start(
                            l32[:, s * CPS:(s + 1) * CPS],
                            l_dram[qsl].rearrange("(i p) -> p i", p=P))
                        nc.vector.reciprocal(recip_l[:, s * CPS:(s + 1) * CPS],
                                             l32[:, s * CPS:(s + 1) * CPS])

                # ---- Phase D: projection + residual + relu + store ----
                with (
                    tc.tile_pool(name="ps_d", bufs=4, space="PSUM") as ps_d,
                    tc.tile_pool(name="res", bufs=4) as respool,
                ):
                    for i in range(NCH):
                        pp = ps_d.tile([P, C], F32, tag="pp")
                        nc.tensor.matmul(pp[:], attT[:, ts(i, P)], wp_bf[:],
                                         start=True, stop=True)
                        res = respool.tile([P, C], F32, tag="res")
                        nc.vector.scalar_tensor_tensor(
                            res[:], pp[:], recip_l[:, i:i + 1],
                            x_nat[:, i, :], op0=OP.mult, op1=OP.add)
                        nc.gpsimd.tensor_relu(res[:], res[:])
                        nc.sync.dma_start(out_d[ts(i, P), :], res[:])

            if hw_loop is not None:
                with tc.For_i(0, hw_loop) as _i:
                    emit(0)
            else:
                for _rep in range(reps):
                    emit(_rep)

    nc.finalize()
    return nc


_cached_nc = None


def _make_in_maps(x, wq, bq, wk, bk, wv, bv, wp, bp):
    cvec = (bv.astype(np.float64) @ wp.astype(np.float64)
            + bp.astype(np.float64)).astype(np.float32).reshape(1, C)
    in_maps = []
    for b in range(B):
        in_maps.append({
            "x": np.ascontiguousarray(x[b].reshape(NT, C), dtype=np.float32),
            "wq": np.ascontiguousarray(wq, dtype=np.float32),
            "wk": np.ascontiguousarray(wk, dtype=np.float32),
            "wv": np.ascontiguousarray(wv, dtype=np.float32),
            "wp": np.ascontiguousarray(wp, dtype=np.float32),
            "bq": np.ascontiguousarray(bq.reshape(HC, 1), dtype=np.float32),
            "bk": np.ascontiguousarray(bk.reshape(HC, 1), dtype=np.float32),
            "cvec": cvec,
        })
    return in_maps


def kernel(x, wq, bq, wk, bk, wv, bv, wp, bp):
    global _cached_nc
    x = np.asarray(x)
    if _cached_nc is None:
        _cached_nc = build()
    in_maps = _make_in_maps(np.asarray(x), np.asarray(wq), np.asarray(bq),
                            np.asarray(wk), np.asarray(bk), np.asarray(wv),
                            np.asarray(bv), np.asarray(wp), np.asarray(bp))
    res = run_bass_kernel_spmd(_cached_nc, in_maps, core_ids=list(range(B)))
    out = np.stack([res.results[b]["out"].reshape(H, W, C) for b in range(B)])
    return out.astype(np.float32)
